# revision 8
# baseline (speedup 1.0000x reference)
"""Bass/Tile kernel: cosine top-20 adjacency (16384x64 embeddings) on 8 trn2 cores.

Per-core algorithm (rows sharded 2048/core via host-side input rotation, so the
same SPMD graph runs on every core):
  1. Load embeddings row-major, compute row norms (square -> windowed reduce ->
     sqrt), then a fused normalize+bf16-cast (divide by broadcast norm).
  2. Round-trip the normalized bf16 matrix through DRAM and XBAR-transpose its
     [8192, 128] view -> normT [64, 16384] with columns permuted to
     [even rows | odd rows] (column order is irrelevant: output is values-only).
  3. Per 128-row tile: sim = lhsT.T @ normT (bf16 matmuls into PSUM), PSUM
     evacuated via {Act bf16 cast | DVE fused max-fold}, bf16 max-fold pyramid
     (DVE 4x + gpsimd) to 512 windowed maxima, max8 per 128-chunk -> 32
     candidates, 3x(max8 + match_replace) -> top-24 descending.
  4. Self-similarity (~1.0) is always the strict row max, so
     out[:,0] = 0 and out[:,1:20] = sigmoid(top24[:,1:20]).
"""

import os
import sys

import numpy as np

for _p in ("/opt/trn_rl_repo",):
    if _p not in sys.path and os.path.isdir(_p):
        sys.path.insert(0, _p)

import concourse.bass as bass  # noqa: E402
import concourse.mybir as mybir  # noqa: E402
import concourse.tile as tile  # noqa: E402
from concourse import bacc  # noqa: E402
from concourse.bass_utils import run_bass_kernel_spmd  # noqa: E402

N = 16384
D = 64
TOPK = 20
CORES = 8
R = N // CORES  # 2048 rows per core
T = R // 128  # 16 row tiles per core
G = 2048  # column group size
NG = N // G  # 8 column groups
H = N // 2  # even/odd half size in permuted column space
NEG = -1.0e30

f32 = mybir.dt.float32
bf16 = mybir.dt.bfloat16
AF = mybir.ActivationFunctionType
ALU = mybir.AluOpType

# Evacuation per 2048-col group (HW: only ONE instruction input may be PSUM,
# gpsimd cannot run TensorTensor/stt at all):
#   A-path (g < NACT): Act casts the full group to bf16; DVE folds it at 4x.
#   H-path (else): Act casts the hi half; DVE folds psum-lo against it at 1x.
NACT = 4

_CACHE = {}


def _stt_max(nc, eng, out_ap, in0_ap, in1_ap):
    getattr(nc, eng).scalar_tensor_tensor(
        out_ap, in0_ap, 1.0, in1_ap, op0=ALU.mult, op1=ALU.max
    )


def _build_nc():
    nc = bacc.Bacc(
        "TRN2", target_bir_lowering=False, debug=False, enable_asserts=False
    )
    emb = nc.dram_tensor("embeddings", [N, D], f32, kind="ExternalInput")
    out = nc.dram_tensor("out", [R, TOPK], f32, kind="ExternalOutput")
    # out rows within a tile come out parity-permuted: local row = t*128 + 2q + h
    # for osb partition m = h*64 + q. This view un-permutes in the output DMA.
    out_v = out[:].rearrange("(t q h) k -> t h q k", q=64, h=2)

    with tile.TileContext(nc) as tc:
        with tc.tile_pool(name="persist", bufs=1) as persist:
            normT = persist.tile([D, N], bf16)

            # ---- Prologue: normalize rows, cast bf16, XBAR transpose ----
            with (
                tc.tile_pool(name="pro_rm", bufs=1) as pro_rm,
                tc.tile_pool(name="pro_t2", bufs=1) as pro_t2,
                tc.tile_pool(name="pro_dram", bufs=1, space="DRAM") as pro_dram,
            ):
                # flat [128, 8192] staging view of the [16384, 64] layout:
                # partition p holds rows p*128 .. p*128+127 (row r = p*128 + a)
                emb_flat = emb[:].rearrange("(p a) d -> p (a d)", p=128)
                rm = pro_rm.tile([128, 128, D], f32)
                nc.sync.dma_start(
                    rm[:, 0:64, :], emb_flat[:, 0:4096].rearrange(
                        "p (a d) -> p a d", d=D
                    ),
                )
                nc.scalar.dma_start(
                    rm[:, 64:128, :], emb_flat[:, 4096:8192].rearrange(
                        "p (a d) -> p a d", d=D
                    ),
                )
                sq = pro_rm.tile([128, 128, D], f32)
                nc.scalar.activation(sq[:], rm[:], AF.Square)
                ssq = pro_rm.tile([128, 128], f32)
                nc.vector.tensor_reduce(
                    ssq[:], sq[:], axis=mybir.AxisListType.X, op=ALU.add
                )
                slen = pro_rm.tile([128, 128], f32)
                nc.scalar.activation(slen[:], ssq[:], AF.Sqrt)
                sinv = pro_rm.tile([128, 128], f32)
                nc.vector.reciprocal(sinv[:], slen[:])
                # fused normalize + bf16 cast: rmb = rm * sinv (broadcast over d)
                rmb = pro_rm.tile([128, 128, D], bf16)
                nc.vector.scalar_tensor_tensor(
                    rmb[:], rm[:], 1.0, sinv[:].to_broadcast((128, 128, D)),
                    op0=ALU.mult, op1=ALU.mult,
                )

                scratch = pro_dram.tile([N, D], bf16)
                sc_flat = scratch[:].rearrange("(p a) d -> p a d", p=128)
                nc.sync.dma_start(sc_flat[:, 0:64, :], rmb[:, 0:64, :])
                nc.scalar.dma_start(sc_flat[:, 64:128, :], rmb[:, 64:128, :])

                # XBAR transpose of the [8192, 128] bf16 view: nt2 partition
                # c<64 holds column c over even rows; 64+d over odd rows.
                sc_v = scratch[:].rearrange("(m two) d -> m (two d)", two=2)
                nt2 = pro_t2.tile([128, H], bf16)
                nc.scalar.dma_start(out=nt2[:], in_=sc_v, transpose=True)

                nc.gpsimd.tensor_copy(normT[:, 0:H], nt2[0:D, :])
                nc.sync.dma_start(normT[:, H:N], nt2[D:128, :])

            # lhsT for row tile t: local rows r = t*128 + 2q + h live at
            # permuted column h*8192 + t*64 + q -> AP [64, 2, 64], free order
            # (h, q) = out partition m = h*64 + q.
            normT_v = normT[:].rearrange("d (h j) -> d h j", h=2)

            # ---- Main loop: 16 row tiles ----
            with (
                tc.tile_pool(name="mm_psum", bufs=2, space="PSUM") as mm_psum,
                tc.tile_pool(name="ev_cast", bufs=6) as ev_cast,
                tc.tile_pool(name="pyr", bufs=2) as pyr,
                tc.tile_pool(name="fin", bufs=2) as fin,
            ):
                for t in range(T):
                    lhsT = fin.tile([D, 128], bf16, tag="lhsT")
                    nc.gpsimd.tensor_copy(
                        lhsT[:], normT_v[:, :, t * 64 : (t + 1) * 64]
                    )
                    l1b = pyr.tile([128, NG, G // 2], bf16, tag="l1b")
                    for g in range(NG):
                        ps = mm_psum.tile([128, G], f32, tag="ps")
                        for s in range(G // 512):
                            cs = slice(g * G + s * 512, g * G + (s + 1) * 512)
                            nc.tensor.matmul(
                                ps[:, s * 512 : (s + 1) * 512],
                                lhsT[:],
                                normT[:, cs],
                            )
                        if g < NACT:
                            cast_g = ev_cast.tile([128, G], bf16, tag="cast")
                            nc.scalar.activation(cast_g[:], ps[:], AF.Copy)
                            _stt_max(
                                nc, "vector", l1b[:, g, :],
                                cast_g[:, 0 : G // 2], cast_g[:, G // 2 : G],
                            )
                        else:  # H-path: Act casts hi half, DVE folds lo vs it
                            casth_g = ev_cast.tile(
                                [128, G // 2], bf16, tag="casth"
                            )
                            nc.scalar.activation(
                                casth_g[:], ps[:, G // 2 : G], AF.Copy
                            )
                            _stt_max(
                                nc, "vector", l1b[:, g, :],
                                ps[:, 0 : G // 2], casth_g[:],
                            )

                    # fold pyramid: 8x1024 -> 4x1024 -> 2x1024 -> 1024 -> 512
                    l2b = pyr.tile([128, 4, G // 2], bf16, tag="l2b")
                    _stt_max(nc, "vector", l2b[:], l1b[:, 0:4, :], l1b[:, 4:8, :])
                    l3b = pyr.tile([128, 2, G // 2], bf16, tag="l3b")
                    _stt_max(nc, "vector", l3b[:], l2b[:, 0:2, :], l2b[:, 2:4, :])
                    l4 = pyr.tile([128, G // 2], bf16, tag="l4")
                    _stt_max(nc, "vector", l4[:], l3b[:, 0, :], l3b[:, 1, :])
                    l5 = pyr.tile([128, G // 4], bf16, tag="l5")
                    _stt_max(
                        nc, "vector", l5[:],
                        l4[:, 0 : G // 4], l4[:, G // 4 : G // 2],
                    )

                    # candidates: top-8 of each 128-chunk of the 512 maxima
                    cand = fin.tile([128, 32], bf16, tag="cand")
                    for c in range(4):
                        nc.vector.max(
                            out=cand[:, c * 8 : (c + 1) * 8],
                            in_=l5[:, c * 128 : (c + 1) * 128],
                        )
                    # top-24 via 3x max8 + 2x match_replace
                    top24 = fin.tile([128, 24], bf16, tag="top24")
                    cand2 = fin.tile([128, 32], bf16, tag="cand2")
                    cand3 = fin.tile([128, 32], bf16, tag="cand3")
                    nc.vector.max(out=top24[:, 0:8], in_=cand[:])
                    nc.vector.match_replace(
                        out=cand2[:], in_to_replace=top24[:, 0:8],
                        in_values=cand[:], imm_value=NEG,
                    )
                    nc.vector.max(out=top24[:, 8:16], in_=cand2[:])
                    nc.vector.match_replace(
                        out=cand3[:], in_to_replace=top24[:, 8:16],
                        in_values=cand2[:], imm_value=NEG,
                    )
                    nc.vector.max(out=top24[:, 16:24], in_=cand3[:])

                    # epilogue: out[:,0] = 0; out[:,1:20] = sigmoid(top24[:,1:20])
                    osb = fin.tile([128, TOPK], f32, tag="osb")
                    nc.gpsimd.memset(osb[:, 0:1], 0.0)
                    nc.scalar.activation(
                        osb[:, 1:TOPK], top24[:, 1:TOPK], AF.Sigmoid
                    )
                    nc.sync.dma_start(out_v[t], osb[:])

    nc.compile()
    return nc


def get_nc():
    if "nc" not in _CACHE:
        _CACHE["nc"] = _build_nc()
    return _CACHE["nc"]


def kernel(embeddings: np.ndarray) -> np.ndarray:
    emb = np.ascontiguousarray(np.asarray(embeddings, dtype=np.float32))
    assert emb.shape == (N, D), emb.shape
    nc = get_nc()
    in_maps = [
        {"embeddings": np.roll(emb, -i * R, axis=0)} for i in range(CORES)
    ]
    res = run_bass_kernel_spmd(nc, in_maps, core_ids=list(range(CORES)))
    _CACHE["last_results"] = res
    return np.concatenate(
        [res.results[i]["out"] for i in range(CORES)], axis=0
    ).astype(np.float32)


# revision 10
# speedup vs baseline: 1.2156x; 1.2156x over previous
"""Bass/Tile kernel: cosine top-20 adjacency (16384x64 embeddings) on 8 trn2 cores.

Per-core algorithm (rows sharded 2048/core via host-side input rotation, so the
same SPMD graph runs on every core):
  1. Load embeddings row-major, compute row norms (square -> windowed reduce ->
     sqrt), then a fused normalize+bf16-cast (divide by broadcast norm).
  2. Round-trip the normalized bf16 matrix through DRAM and XBAR-transpose its
     [8192, 128] view -> normT [64, 16384] with columns permuted to
     [even rows | odd rows] (column order is irrelevant: output is values-only).
  3. Per 128-row tile: sim = lhsT.T @ normT (bf16 matmuls into PSUM), PSUM
     evacuated via {Act bf16 cast | DVE fused max-fold}, bf16 max-fold pyramid
     (DVE 4x + gpsimd) to 512 windowed maxima, max8 per 128-chunk -> 32
     candidates, 3x(max8 + match_replace) -> top-24 descending.
  4. Self-similarity (~1.0) is always the strict row max, so
     out[:,0] = 0 and out[:,1:20] = sigmoid(top24[:,1:20]).
"""

import os
import sys

import numpy as np

for _p in ("/opt/trn_rl_repo",):
    if _p not in sys.path and os.path.isdir(_p):
        sys.path.insert(0, _p)

import concourse.bass as bass  # noqa: E402
import concourse.mybir as mybir  # noqa: E402
import concourse.tile as tile  # noqa: E402
from concourse import bacc  # noqa: E402
from concourse.bass_utils import run_bass_kernel_spmd  # noqa: E402

N = 16384
D = 64
TOPK = 20
CORES = 8
R = N // CORES  # 2048 rows per core
T = R // 128  # 16 row tiles per core
G = 2048  # column group size
NG = N // G  # 8 column groups
H = N // 2  # even/odd half size in permuted column space
NEG = -1.0e30

f32 = mybir.dt.float32
bf16 = mybir.dt.bfloat16
AF = mybir.ActivationFunctionType
ALU = mybir.AluOpType

# Evacuation per 2048-col group (HW: only ONE instruction input may be PSUM,
# gpsimd cannot run TensorTensor/stt at all):
#   A-path (g < NACT): Act casts the full group to bf16; DVE folds it at 4x.
#   H-path (else): Act casts the hi half; DVE folds psum-lo against it at 1x.
NACT = 4

_CACHE = {}


def _stt_max(nc, eng, out_ap, in0_ap, in1_ap):
    getattr(nc, eng).scalar_tensor_tensor(
        out_ap, in0_ap, 1.0, in1_ap, op0=ALU.mult, op1=ALU.max
    )


def _build_nc():
    nc = bacc.Bacc(
        "TRN2", target_bir_lowering=False, debug=False, enable_asserts=False
    )
    emb = nc.dram_tensor("embeddings", [N, D], f32, kind="ExternalInput")
    out = nc.dram_tensor("out", [R, TOPK], f32, kind="ExternalOutput")
    # lhsT for tile t is the contiguous permuted-column slice [t*128, t*128+128),
    # which corresponds to local rows 2*(t*128+q) for t<8 (even rows) and
    # 2*((t-8)*128+q)+1 for t>=8 (odd rows). This view un-permutes on output DMA:
    # out_v[h, j] = local row 2j+h.
    out_v = out[:].rearrange("(j two) k -> two j k", two=2)

    with tile.TileContext(nc) as tc:
        with tc.tile_pool(name="persist", bufs=1) as persist:
            normT = persist.tile([D, N], bf16)

            # ---- Prologue: normalize rows, cast bf16, XBAR transpose ----
            with (
                tc.tile_pool(name="pro_rm", bufs=1) as pro_rm,
                tc.tile_pool(name="pro_t2", bufs=1) as pro_t2,
                tc.tile_pool(name="pro_dram", bufs=1, space="DRAM") as pro_dram,
            ):
                # flat [128, 8192] staging view of the [16384, 64] layout:
                # partition p holds rows p*128 .. p*128+127 (row r = p*128 + a)
                emb_flat = emb[:].rearrange("(p a) d -> p (a d)", p=128)
                rm = pro_rm.tile([128, 128, D], f32)
                nc.sync.dma_start(
                    rm[:, 0:64, :], emb_flat[:, 0:4096].rearrange(
                        "p (a d) -> p a d", d=D
                    ),
                )
                nc.scalar.dma_start(
                    rm[:, 64:128, :], emb_flat[:, 4096:8192].rearrange(
                        "p (a d) -> p a d", d=D
                    ),
                )
                sq = pro_rm.tile([128, 128, D], f32)
                nc.scalar.activation(sq[:], rm[:], AF.Square)
                ssq = pro_rm.tile([128, 128], f32)
                nc.vector.tensor_reduce(
                    ssq[:], sq[:], axis=mybir.AxisListType.X, op=ALU.add
                )
                slen = pro_rm.tile([128, 128], f32)
                nc.scalar.activation(slen[:], ssq[:], AF.Sqrt)
                sinv = pro_rm.tile([128, 128], f32)
                nc.vector.reciprocal(sinv[:], slen[:])
                # fused normalize + bf16 cast: rmb = rm * sinv (broadcast over d)
                rmb = pro_rm.tile([128, 128, D], bf16)
                nc.vector.scalar_tensor_tensor(
                    rmb[:], rm[:], 1.0, sinv[:].to_broadcast((128, 128, D)),
                    op0=ALU.mult, op1=ALU.mult,
                )

                scratch = pro_dram.tile([N, D], bf16)
                sc_flat = scratch[:].rearrange("(p a) d -> p a d", p=128)
                nc.sync.dma_start(sc_flat[:, 0:64, :], rmb[:, 0:64, :])
                nc.scalar.dma_start(sc_flat[:, 64:128, :], rmb[:, 64:128, :])

                # XBAR transpose of the [8192, 128] bf16 view: nt2 partition
                # c<64 holds column c over even rows; 64+d over odd rows.
                sc_v = scratch[:].rearrange("(m two) d -> m (two d)", two=2)
                nt2 = pro_t2.tile([128, H], bf16)
                nc.scalar.dma_start(out=nt2[:], in_=sc_v, transpose=True)

                nc.gpsimd.tensor_copy(normT[:, 0:H], nt2[0:D, :])
                nc.sync.dma_start(normT[:, H:N], nt2[D:128, :])

            # ---- Main loop: 16 row tiles ----
            with (
                tc.tile_pool(name="mm_psum", bufs=2, space="PSUM") as mm_psum,
                tc.tile_pool(name="ev_cast", bufs=2) as ev_cast,
                tc.tile_pool(name="pyr", bufs=2) as pyr,
                tc.tile_pool(name="pyr1", bufs=1) as pyr1,
                tc.tile_pool(name="fin", bufs=2) as fin,
            ):
                for t in range(T):
                    # tile t<8: even local rows (permuted cols t*128..);
                    # tile t>=8: odd local rows (permuted cols H+(t-8)*128..)
                    c0 = t * 128 if t < 8 else H + (t - 8) * 128
                    lhsT = normT[:, c0 : c0 + 128]
                    castb = ev_cast.tile([128, N], bf16, tag="castb")
                    for g in range(NG):
                        ps = mm_psum.tile([128, G], f32, tag="ps")
                        for s in range(G // 512):
                            cs = slice(g * G + s * 512, g * G + (s + 1) * 512)
                            nc.tensor.matmul(
                                ps[:, s * 512 : (s + 1) * 512],
                                lhsT,
                                normT[:, cs],
                            )
                        nc.scalar.activation(
                            castb[:, g * G : (g + 1) * G], ps[:], AF.Copy
                        )

                    # fold chain: 16384 -> 8192 -> 4096 -> 2048 -> 1024 -> 512
                    f1 = pyr.tile([128, N // 2], bf16, tag="f1")
                    nc.vector.tensor_max(
                        f1[:], castb[:, 0 : N // 2], castb[:, N // 2 : N]
                    )
                    f2 = pyr1.tile([128, N // 4], bf16, tag="f2")
                    nc.vector.tensor_max(
                        f2[:], f1[:, 0 : N // 4], f1[:, N // 4 : N // 2]
                    )
                    f3 = pyr1.tile([128, N // 8], bf16, tag="f3")
                    nc.vector.tensor_max(
                        f3[:], f2[:, 0 : N // 8], f2[:, N // 8 : N // 4]
                    )
                    f4 = pyr1.tile([128, N // 16], bf16, tag="f4")
                    nc.vector.tensor_max(
                        f4[:], f3[:, 0 : N // 16], f3[:, N // 16 : N // 8]
                    )
                    f5 = pyr1.tile([128, N // 32], bf16, tag="f5")
                    nc.vector.tensor_max(
                        f5[:], f4[:, 0 : N // 32], f4[:, N // 32 : N // 16]
                    )

                    # candidates: top-8 of each 128-chunk of the 512 maxima
                    cand = fin.tile([128, 32], bf16, tag="cand")
                    for c in range(4):
                        nc.vector.max(
                            out=cand[:, c * 8 : (c + 1) * 8],
                            in_=f5[:, c * 128 : (c + 1) * 128],
                        )
                    # top-24 via 3x max8 + 2x match_replace
                    top24 = fin.tile([128, 24], bf16, tag="top24")
                    cand2 = fin.tile([128, 32], bf16, tag="cand2")
                    cand3 = fin.tile([128, 32], bf16, tag="cand3")
                    nc.vector.max(out=top24[:, 0:8], in_=cand[:])
                    nc.vector.match_replace(
                        out=cand2[:], in_to_replace=top24[:, 0:8],
                        in_values=cand[:], imm_value=NEG,
                    )
                    nc.vector.max(out=top24[:, 8:16], in_=cand2[:])
                    nc.vector.match_replace(
                        out=cand3[:], in_to_replace=top24[:, 8:16],
                        in_values=cand2[:], imm_value=NEG,
                    )
                    nc.vector.max(out=top24[:, 16:24], in_=cand3[:])

                    # epilogue: out[:,0] = 0; out[:,1:20] = sigmoid(top24[:,1:20])
                    osb = fin.tile([128, TOPK], f32, tag="osb")
                    nc.gpsimd.memset(osb[:, 0:1], 0.0)
                    nc.scalar.activation(
                        osb[:, 1:TOPK], top24[:, 1:TOPK], AF.Sigmoid
                    )
                    h, band = (0, t) if t < 8 else (1, t - 8)
                    nc.sync.dma_start(
                        out_v[h, band * 128 : (band + 1) * 128, :], osb[:]
                    )

    nc.compile()
    return nc


def get_nc():
    if "nc" not in _CACHE:
        _CACHE["nc"] = _build_nc()
    return _CACHE["nc"]


def kernel(embeddings: np.ndarray) -> np.ndarray:
    emb = np.ascontiguousarray(np.asarray(embeddings, dtype=np.float32))
    assert emb.shape == (N, D), emb.shape
    nc = get_nc()
    in_maps = [
        {"embeddings": np.roll(emb, -i * R, axis=0)} for i in range(CORES)
    ]
    res = run_bass_kernel_spmd(nc, in_maps, core_ids=list(range(CORES)))
    _CACHE["last_results"] = res
    return np.concatenate(
        [res.results[i]["out"] for i in range(CORES)], axis=0
    ).astype(np.float32)
